# revision 13
# baseline (speedup 1.0000x reference)
"""Trainium2 Bass kernel for the DNL (disentangled non-local) attention block.

Reference computation (per batch b, with xf = x.reshape(B, C, N), N = H*W):
    q  = (wq @ xf + bq)  centered over n          [N, 32]
    k  = (wk @ xf + bk)  centered over n          [32, N]
    A  = softmax_rows(q @ k)                      [N, N]
    v  = relu(wv @ xf + bv)                       [C, N]
    mask = softmax(wm @ xf + bm)                  [N]
    tissue[c, m] = sum_n v[c, n] * (A[m, n] + mask[n])
    return (x, tissue)

Math simplifications used (all exact):
  - q/k biases, bm, and k-centering add per-row constants inside the row
    softmax and drop out; only q-centering survives (as "-mean_n q").
  - The mask term is a rank-1 correction vm[c] = sum_n v[c,n] mask[n].
  - No max-subtraction in softmax: |energy| <= ~5 for these input scales.

Device layout (per core; 8 cores = 4 batches x 2 query-halves of 2048):
  - E^T[j, m] = K[:, j]^T @ Qc^T[:, m] computed j-partitioned so that the
    softmax denominator (colsum) and the AV matmul both consume it without
    any transposes.  O[c, m] = V^T-blocks^T @ expE^T accumulates in PSUM,
    with the denominator colsum as a third ones-column matmul pass per exp
    tile (keeps the whole reduction on the PE; GpSimd adds measured ~2.7x
    below nominal and starved the PE).
  - The per-core query half is selected by permuting the spatial columns of
    the input on the host (j-sums are permutation invariant).

Schedule notes (from perfetto traces):
  - Weight DMAs are emitted BEFORE the 4MB x DMA: everything funnels
    through one DMA queue, and weights behind x delayed the first matmul
    by ~13us.
  - K and Q project in one packed pass (stationary [128, 64] = wk|wq);
    q-sums for centering fall out of the ACT staging copies via accum_out.
  - pm (mask logits) is computed directly in column layout [128, NB] by
    reusing the V-projection's stationary xf blocks with wm as the moving
    operand (N=1 matmuls) - no transposes anywhere.
  - The per-m-chunk epilogue (1/s, *rs, +vm, DMA out) contains zero PE
    instructions (reciprocal_approx_fast on DVE + partition_broadcast on
    GpSimd) and is emitted lazily one chunk later so the PE FIFO never
    waits on it; only the PSUM-freeing copies are emitted eagerly.
"""

import sys

import numpy as np

if "/opt/trn_rl_repo" not in sys.path:
    sys.path.insert(0, "/opt/trn_rl_repo")

import concourse.bacc as bacc
import concourse.bass as bass
import concourse.mybir as mybir
import concourse.tile as tile
from concourse.bass_utils import run_bass_kernel_spmd

F32 = mybir.dt.float32
F32R = mybir.dt.float32r
BF16 = mybir.dt.bfloat16
AF = mybir.ActivationFunctionType

B, C, H, W = 4, 256, 64, 64
N = H * W          # 4096 spatial positions
D = 32             # C // 8, q/k channel dim
M = N // 2         # query rows per core (2048)
NB = N // 128      # 32 j-blocks
NMC = M // 512     # 4 m-chunks per core
NG = NB // 2       # 16 2-jb groups per m-chunk
N_CORES = 8
C_SPLITS = [(0, 128), (128, 128)]

# fp32r = fp32 bit layout, reduced-precision PE path (1 cyc/row vs 4 for
# fp32 when the moving dim >= 256).  walrus requires every fp32r-matmul
# operand's memory location to be typed float32r, so all matmul-feeding
# tiles below are F32R; compute-engine writes round into them.


def build_nc():
    nc = bacc.Bacc("TRN2", target_bir_lowering=False)

    x_d = nc.dram_tensor("x", [C, N], F32R, kind="ExternalInput")
    wkq_d = nc.dram_tensor("wkq", [128, 2, 2 * D + 1], F32R, kind="ExternalInput")
    wvt_d = nc.dram_tensor("wvt", [128, 2, C], F32R, kind="ExternalInput")
    bv_d = nc.dram_tensor("bv", [1, C], F32R, kind="ExternalInput")
    out_d = nc.dram_tensor("out", [C, M], F32, kind="ExternalOutput")

    with tile.TileContext(nc) as tc, nc.allow_low_precision(
        reason="fp32r/bf16 matmul operands are a deliberate precision trade"
    ):
        with (
            tc.tile_pool(name="const", bufs=1) as cpool,
            tc.tile_pool(name="work", bufs=1) as wpool,
            tc.tile_pool(name="norm", bufs=2) as npool,
            tc.tile_pool(name="expsb", bufs=3) as epool,
            tc.tile_pool(name="osb", bufs=2) as opool,
        ):
            # ---------------- DMAs: small weights first ----------------
            wkq = cpool.tile([128, 2, 2 * D + 1], F32R, tag="wkq")
            wvt = cpool.tile([128, 2, C], F32R, tag="wvt")
            bv = cpool.tile([1, C], F32R, tag="bv")
            nc.sync.dma_start(wkq[:], wkq_d[:])
            nc.sync.dma_start(bv[:], bv_d[:])

            xf0 = cpool.tile([128, N], F32R, tag="xf0")
            xf1 = cpool.tile([128, N], F32R, tag="xf1")
            nc.sync.dma_start(xf0[:, bass.ts(0, 1024)], x_d[0:128, bass.ts(0, 1024)])
            nc.sync.dma_start(xf1[:, bass.ts(0, 1024)], x_d[128:256, bass.ts(0, 1024)])
            nc.sync.dma_start(wvt[:], wvt_d[:])
            for t in range(1, 4):
                nc.sync.dma_start(
                    xf0[:, bass.ts(t, 1024)], x_d[0:128, bass.ts(t, 1024)]
                )
                nc.sync.dma_start(
                    xf1[:, bass.ts(t, 1024)], x_d[128:256, bass.ts(t, 1024)]
                )
            xfs = [xf0, xf1]

            ones_colf = cpool.tile([128, 1], F32, tag="ones_colf")
            nc.vector.memset(ones_colf[:], 1.0)
            ones_col = cpool.tile([128, 1], BF16, tag="ones_col")
            nc.vector.tensor_copy(ones_col[:], ones_colf[:])
            ones_rowf = cpool.tile([1, 128], F32, tag="ones_rowf")
            nc.vector.memset(ones_rowf[:], 1.0)
            ones_row = cpool.tile([1, 128], F32R, tag="ones_row")
            nc.vector.tensor_copy(ones_row[:], ones_rowf[:])

            # ---------------- stage A: projections ----------------
            k_sb = cpool.tile([2 * D, N], BF16, tag="k_sb")
            qt_sb = cpool.tile([2 * D, N], F32, tag="qt_sb")
            qct = cpool.tile([2 * D, M], BF16, tag="qct")
            qpart = cpool.tile([2 * D, 8], F32, tag="qpart")
            vt_sb = cpool.tile([128, NB * C], BF16, tag="vt_sb")
            mask_col = cpool.tile([128, NB], BF16, tag="mask_col")
            emask_row = cpool.tile([1, N], F32, tag="emask_row")
            ztpart = cpool.tile([1, 8], F32, tag="ztpart")
            ztt = cpool.tile([1, 1], F32, tag="ztt")
            rz = cpool.tile([1, 1], F32, tag="rz")
            rzc = cpool.tile([128, 1], F32, tag="rzc")
            vm_col = cpool.tile([128, 2], F32, tag="vm_col")
            qsum = wpool.tile([2 * D, 1], F32, tag="qsum")
            qneg = wpool.tile([2 * D, 1], F32, tag="qneg")

            with (
                tc.tile_pool(name="psA", bufs=2, space="PSUM") as psA,
                tc.tile_pool(name="psB", bufs=2, space="PSUM") as psB,
                tc.tile_pool(name="psT", bufs=1, space="PSUM") as psT,
            ):
                # K|Q|pm packed pass (stationary cols 0-31 = wk,
                # 32-63 = wq, 64 = wm) interleaved with the V projection
                # 4 j-blocks per t-chunk: matches the x DMA arrival pace
                # so the PE never idles during the input stream.
                for t in range(8):
                    kq = psA.tile([2 * D + 1, 512], F32, tag="kq_ps")
                    for cb in range(2):
                        nc.tensor.matmul(
                            kq[:],
                            wkq[:, cb, :],
                            xfs[cb][:, bass.ts(t, 512)],
                            start=(cb == 0),
                            stop=(cb == 1),
                        )
                    nc.scalar.copy(k_sb[0:D, bass.ts(t, 512)], kq[0:D, :])
                    # q staging copy; its free-axis accum gives the q colsum
                    nc.scalar.activation(
                        qt_sb[D : 2 * D, bass.ts(t, 512)],
                        kq[D : 2 * D, :],
                        AF.Copy,
                        accum_out=qpart[D : 2 * D, t : t + 1],
                    )
                    # mask logits: exp straight out of PSUM, with the
                    # softmax denominator accumulating for free
                    nc.scalar.activation(
                        emask_row[0:1, bass.ts(t, 512)],
                        kq[2 * D : 2 * D + 1, :],
                        AF.Exp,
                        accum_out=ztpart[0:1, t : t + 1],
                    )
                    # V^T[n, c] = relu(xf^T @ wv^T + bv), j-block-major
                    for jb in range(4 * t, 4 * t + 4):
                        vp = psB.tile([128, C], F32, tag="v_ps")
                        for cb in range(2):
                            nc.tensor.matmul(
                                vp[:],
                                xfs[cb][:, bass.ts(jb, 128)],
                                wvt[:, cb, :],
                                start=(cb == 0),
                                stop=False,
                            )
                        nc.tensor.matmul(
                            vp[:], ones_row[:], bv[:], start=False, stop=True
                        )
                        nc.vector.tensor_scalar_max(
                            vt_sb[:, bass.ts(jb, C)], vp[:], 0.0
                        )

                # center q over n:  qc = q - mean_n(q)  (rows 32-63).
                # The add runs on GpSimd so the DVE relu-max stream never
                # delays it.
                nc.vector.reduce_sum(
                    qsum[D : 2 * D], qpart[D : 2 * D, :], axis=mybir.AxisListType.X
                )
                nc.scalar.mul(qneg[D : 2 * D], qsum[D : 2 * D], -1.0 / N)
                nc.gpsimd.tensor_scalar_add(
                    qct[D : 2 * D, :], qt_sb[D : 2 * D, 0:M], qneg[D : 2 * D]
                )
                # replicate K and Qc to the other 32-row group for the
                # 2-way row-packed E^T matmuls
                nc.sync.dma_start(k_sb[D : 2 * D, :], k_sb[0:D, :])
                nc.sync.dma_start(qct[0:D, :], qct[D : 2 * D, :])

                # unnormalized mask as a column tile [128, NB]: 32 tiny PE
                # transposes of [1, 128] slices (internal DRAM tiles fail
                # to load on this runtime, so no DRAM round trip)
                mcp = psT.tile([128, NB], F32, tag="mc_ps")
                for jb in range(NB):
                    nc.tensor.transpose(
                        mcp[:, jb : jb + 1],
                        emask_row[0:1, bass.ts(jb, 128)],
                        ones_rowf[0:1, 0:1],
                    )
                nc.vector.tensor_copy(mask_col[:], mcp[:])
                nc.vector.reduce_sum(
                    ztt[:], ztpart[:], axis=mybir.AxisListType.X
                )
                nc.vector.reciprocal_approx_fast(rz[:], ztt[:])
                nc.gpsimd.partition_broadcast(rzc[:], rz[:])

            # ---------------- main loop: attention ----------------
            with (
                tc.tile_pool(name="psE", bufs=2, space="PSUM") as psE,
                tc.tile_pool(name="psO", bufs=1, space="PSUM") as psO,
                tc.tile_pool(name="psS", bufs=1, space="PSUM") as psS,
                tc.tile_pool(name="psVM", bufs=1, space="PSUM") as psVM,
            ):

                def emit_e(mc, g):
                    # two K=32 matmuls packed into row groups (0,0)/(32,0);
                    # they run concurrently in the PE array
                    e_ps = psE.tile([128, 1024], F32, tag="e_ps", name="e_ps")
                    for h in range(2):
                        jb = 2 * g + h
                        nc.tensor.matmul(
                            e_ps[:, bass.ts(h, 512)],
                            k_sb[h * D : (h + 1) * D, bass.ts(jb, 128)],
                            qct[h * D : (h + 1) * D, bass.ts(mc, 512)],
                            start=True,
                            stop=True,
                            tile_position=(h * D, 0),
                        )
                    return e_ps

                def emit_vm():
                    # rank-1 mask correction, accumulated directly as
                    # per-partition columns (N=1 matmuls, no transposes)
                    vm_ps = psVM.tile([128, 2], F32, tag="vm_ps")
                    for jb in range(NB):
                        for h in range(2):
                            # one accumulation group for the whole
                            # bank: a second start=True on the same bank
                            # clears the other column's first contribution
                            nc.tensor.matmul(
                                vm_ps[:, h : h + 1],
                                vt_sb[:, jb * C + h * 128 : jb * C + (h + 1) * 128],
                                mask_col[:, jb : jb + 1],
                                start=(jb == 0 and h == 0),
                                stop=(jb == NB - 1 and h == 1),
                            )
                    nc.vector.tensor_scalar_mul(vm_col[:], vm_ps[:], rzc[:, 0:1])

                def emit_tail(mc, orw0, orw1, s_sb):
                    # normalize + mask add + store; no PE instructions
                    rs = npool.tile([1, 512], F32, tag="rs", name="rs")
                    nc.vector.reciprocal_approx_fast(rs[:], s_sb[:])
                    rb = npool.tile([128, 512], F32, tag="rb", name="rb")
                    nc.gpsimd.partition_broadcast(rb[:], rs[:])
                    for ci, orw, eng in ((0, orw0, nc.vector), (1, orw1, nc.gpsimd)):
                        t_sb = opool.tile([128, 512], F32, tag="t_sb", name="t_sb")
                        eng.tensor_mul(t_sb[:], orw[:], rb[:])
                        o_sb = opool.tile([128, 512], F32, tag="o_sb", name="o_sb")
                        eng.tensor_scalar_add(
                            o_sb[:], t_sb[:], vm_col[:, ci : ci + 1]
                        )
                        nc.sync.dma_start(
                            out_d[ci * 128 : (ci + 1) * 128, bass.ts(mc, 512)],
                            o_sb[:],
                        )

                e_pending = emit_e(0, 0)
                tail_args = None
                for mc in range(NMC):
                    o_ps = [
                        psO.tile([128, 512], F32, tag=f"o_ps{ci}", name=f"o_ps{ci}")
                        for ci in range(2)
                    ]
                    s_ps = psS.tile([1, 512], F32, tag="s_ps", name="s_ps")
                    for g in range(NG):
                        e_sb = epool.tile([128, 1024], BF16, tag="e_sb", name="e_sb")
                        nc.scalar.activation(e_sb[:], e_pending[:], AF.Exp)
                        if g + 1 < NG:
                            e_pending = emit_e(mc, g + 1)
                        elif mc + 1 < NMC:
                            e_pending = emit_e(mc + 1, 0)
                        for h in range(2):
                            jb = 2 * g + h
                            first = jb == 0
                            last = jb == NB - 1
                            e_half = e_sb[:, bass.ts(h, 512)]
                            for ci, (c0, cn) in enumerate(C_SPLITS):
                                nc.tensor.matmul(
                                    o_ps[ci][0:cn, :],
                                    vt_sb[:, jb * C + c0 : jb * C + c0 + cn],
                                    e_half,
                                    start=first,
                                    stop=last,
                                )
                            nc.tensor.matmul(
                                s_ps[:],
                                ones_col[:, 0:1],
                                e_half,
                                start=first,
                                stop=last,
                            )
                        if mc == 1 and g == 0:
                            emit_vm()
                        if g == 1 and tail_args is not None:
                            emit_tail(*tail_args)
                            tail_args = None
                    # eager PSUM-freeing copies so the next chunk's first
                    # accumulations never wait on the (lazy) tail
                    orw0 = opool.tile([128, 512], F32, tag="o_raw0", name="o_raw0")
                    nc.vector.tensor_copy(orw0[:], o_ps[0][:])
                    orw1 = opool.tile([128, 512], F32, tag="o_raw1", name="o_raw1")
                    nc.vector.tensor_copy(orw1[:], o_ps[1][:])
                    s_sb = npool.tile([1, 512], F32, tag="s_sb", name="s_sb")
                    nc.scalar.copy(s_sb[:], s_ps[:])
                    tail_args = (mc, orw0, orw1, s_sb)
                emit_tail(*tail_args)

    nc.compile()
    return nc


_NC_CACHE = {}


def _get_nc():
    if "nc" not in _NC_CACHE:
        _NC_CACHE["nc"] = build_nc()
    return _NC_CACHE["nc"]


def build_in_maps(x, wq, bq, wk, bk, wv, bv, wm, bm):
    x = np.ascontiguousarray(np.asarray(x, dtype=np.float32))
    xf = x.reshape(B, C, N)
    wq = np.asarray(wq, np.float32)
    wk = np.asarray(wk, np.float32)
    wv = np.asarray(wv, np.float32)
    wm = np.asarray(wm, np.float32)
    # packed K|Q|pm stationary: [p, cb, 0:32]=wk.T block, [32:64]=wq.T,
    # [64]=wm.T
    wkq = np.ascontiguousarray(
        np.concatenate(
            [
                wk.T.reshape(2, 128, D),
                wq.T.reshape(2, 128, D),
                wm.T.reshape(2, 128, 1),
            ],
            axis=2,
        ).transpose(1, 0, 2)
    )
    wvt = np.ascontiguousarray(wv.T.reshape(2, 128, C).transpose(1, 0, 2))
    bv2 = np.ascontiguousarray(np.asarray(bv, np.float32).reshape(1, C))

    in_maps = []
    for core in range(N_CORES):
        b, half = divmod(core, 2)
        if half == 0:
            xin = xf[b]
        else:
            # own query half first; j-sums are permutation invariant
            xin = np.concatenate([xf[b][:, M:], xf[b][:, :M]], axis=1)
        in_maps.append(
            {
                "x": np.ascontiguousarray(xin),
                "wkq": wkq,
                "wvt": wvt,
                "bv": bv2,
            }
        )
    return x, in_maps


def kernel(x, wq, bq, wk, bk, wv, bv, wm, bm):
    x, in_maps = build_in_maps(x, wq, bq, wk, bk, wv, bv, wm, bm)

    res = run_bass_kernel_spmd(_get_nc(), in_maps, list(range(N_CORES)))
    _NC_CACHE["last_results"] = res

    tissue = np.empty((B, C, N), np.float32)
    for core in range(N_CORES):
        b, half = divmod(core, 2)
        tissue[b][:, half * M : (half + 1) * M] = res.results[core]["out"]
    return x, tissue.reshape(B, C, H, W)


# revision 14
# speedup vs baseline: 1.1911x; 1.1911x over previous
"""Trainium2 Bass kernel for the DNL (disentangled non-local) attention block.

Reference computation (per batch b, with xf = x.reshape(B, C, N), N = H*W):
    q  = (wq @ xf + bq)  centered over n          [N, 32]
    k  = (wk @ xf + bk)  centered over n          [32, N]
    A  = softmax_rows(q @ k)                      [N, N]
    v  = relu(wv @ xf + bv)                       [C, N]
    mask = softmax(wm @ xf + bm)                  [N]
    tissue[c, m] = sum_n v[c, n] * (A[m, n] + mask[n])
    return (x, tissue)

Math simplifications used (all exact):
  - q/k biases, bm, and k-centering add per-row constants inside the row
    softmax and drop out; only q-centering survives (as "-mean_n q").
  - The mask term is a rank-1 correction vm[c] = sum_n v[c,n] mask[n].
  - No max-subtraction in softmax: |energy| <= ~5 for these input scales.

Device layout (per core; 8 cores = 4 batches x 2 query-halves of 2048):
  - E^T[j, m] = K[:, j]^T @ Qc^T[:, m] computed j-partitioned so that the
    softmax denominator (colsum) and the AV matmul both consume it without
    any transposes.  O[c, m] = V^T-blocks^T @ expE^T accumulates in PSUM,
    with the denominator colsum as a third ones-column matmul pass per exp
    tile (keeps the whole reduction on the PE; GpSimd adds measured ~2.7x
    below nominal and starved the PE).
  - The per-core query half is selected by permuting the spatial columns of
    the input on the host (j-sums are permutation invariant).

Schedule notes (from perfetto traces):
  - Weight DMAs are emitted BEFORE the 4MB x DMA: everything funnels
    through one DMA queue, and weights behind x delayed the first matmul
    by ~13us.
  - K and Q project in one packed pass (stationary [128, 64] = wk|wq);
    q-sums for centering fall out of the ACT staging copies via accum_out.
  - pm (mask logits) is computed directly in column layout [128, NB] by
    reusing the V-projection's stationary xf blocks with wm as the moving
    operand (N=1 matmuls) - no transposes anywhere.
  - The per-m-chunk epilogue (1/s, *rs, +vm, DMA out) contains zero PE
    instructions (reciprocal_approx_fast on DVE + partition_broadcast on
    GpSimd) and is emitted lazily one chunk later so the PE FIFO never
    waits on it; only the PSUM-freeing copies are emitted eagerly.
"""

import sys

import numpy as np

if "/opt/trn_rl_repo" not in sys.path:
    sys.path.insert(0, "/opt/trn_rl_repo")

import concourse.bacc as bacc
import concourse.bass as bass
import concourse.mybir as mybir
import concourse.tile as tile
from concourse.bass_utils import run_bass_kernel_spmd

F32 = mybir.dt.float32
F32R = mybir.dt.float32r
BF16 = mybir.dt.bfloat16
AF = mybir.ActivationFunctionType

B, C, H, W = 4, 256, 64, 64
N = H * W          # 4096 spatial positions
D = 32             # C // 8, q/k channel dim
M = N // 2         # query rows per core (2048)
NB = N // 128      # 32 j-blocks
NMC = M // 512     # 4 m-chunks per core
NG = NB // 2       # 16 2-jb groups per m-chunk
N_CORES = 8
C_SPLITS = [(0, 128), (128, 128)]

# fp32r = fp32 bit layout, reduced-precision PE path (1 cyc/row vs 4 for
# fp32 when the moving dim >= 256).  walrus requires every fp32r-matmul
# operand's memory location to be typed float32r, so all matmul-feeding
# tiles below are F32R; compute-engine writes round into them.


def build_nc():
    nc = bacc.Bacc("TRN2", target_bir_lowering=False)

    x_d = nc.dram_tensor("x", [C, N], F32R, kind="ExternalInput")
    wkq_d = nc.dram_tensor("wkq", [128, 2, 2 * D + 1], F32R, kind="ExternalInput")
    wvt_d = nc.dram_tensor("wvt", [128, 2, C], F32R, kind="ExternalInput")
    bv_d = nc.dram_tensor("bv", [1, C], F32R, kind="ExternalInput")
    out_d = nc.dram_tensor("out", [C, M], F32, kind="ExternalOutput")

    with tile.TileContext(nc) as tc, nc.allow_low_precision(
        reason="fp32r/bf16 matmul operands are a deliberate precision trade"
    ):
        with (
            tc.tile_pool(name="const", bufs=1) as cpool,
            tc.tile_pool(name="work", bufs=1) as wpool,
            tc.tile_pool(name="norm", bufs=2) as npool,
            tc.tile_pool(name="expsb", bufs=3) as epool,
            tc.tile_pool(name="osb", bufs=2) as opool,
        ):
            # ---------------- DMAs: small weights first ----------------
            wkq = cpool.tile([128, 2, 2 * D + 1], F32R, tag="wkq")
            wvt = cpool.tile([128, 2, C], F32R, tag="wvt")
            bv = cpool.tile([1, C], F32R, tag="bv")
            nc.sync.dma_start(wkq[:], wkq_d[:])
            nc.sync.dma_start(bv[:], bv_d[:])

            xf0 = cpool.tile([128, N], F32R, tag="xf0")
            xf1 = cpool.tile([128, N], F32R, tag="xf1")
            nc.sync.dma_start(xf0[:, bass.ts(0, 1024)], x_d[0:128, bass.ts(0, 1024)])
            nc.sync.dma_start(xf1[:, bass.ts(0, 1024)], x_d[128:256, bass.ts(0, 1024)])
            nc.sync.dma_start(wvt[:], wvt_d[:])
            for t in range(1, 4):
                nc.sync.dma_start(
                    xf0[:, bass.ts(t, 1024)], x_d[0:128, bass.ts(t, 1024)]
                )
                nc.sync.dma_start(
                    xf1[:, bass.ts(t, 1024)], x_d[128:256, bass.ts(t, 1024)]
                )
            xfs = [xf0, xf1]

            ones_colf = cpool.tile([128, 1], F32, tag="ones_colf")
            nc.vector.memset(ones_colf[:], 1.0)
            ones_col = cpool.tile([128, 1], BF16, tag="ones_col")
            nc.vector.tensor_copy(ones_col[:], ones_colf[:])
            ones_rowf = cpool.tile([1, 128], F32, tag="ones_rowf")
            nc.vector.memset(ones_rowf[:], 1.0)
            ones_row = cpool.tile([1, 128], F32R, tag="ones_row")
            nc.vector.tensor_copy(ones_row[:], ones_rowf[:])

            # ---------------- stage A: projections ----------------
            k_sb = cpool.tile([2 * D, N], BF16, tag="k_sb")
            qt_sb = cpool.tile([2 * D, N], F32, tag="qt_sb")
            qct = cpool.tile([2 * D, M], BF16, tag="qct")
            qpart = cpool.tile([2 * D, 8], F32, tag="qpart")
            vt_sb = cpool.tile([128, NB * C], BF16, tag="vt_sb")
            mask_col = cpool.tile([128, NB], BF16, tag="mask_col")
            emask_row = cpool.tile([1, N], F32, tag="emask_row")
            ztpart = cpool.tile([1, 8], F32, tag="ztpart")
            ztt = cpool.tile([1, 1], F32, tag="ztt")
            rz = cpool.tile([1, 1], F32, tag="rz")
            rzc = cpool.tile([128, 1], F32, tag="rzc")
            vm_col = cpool.tile([128, 2], F32, tag="vm_col")
            qsum = wpool.tile([2 * D, 1], F32, tag="qsum")
            qneg = wpool.tile([2 * D, 1], F32, tag="qneg")

            with (
                tc.tile_pool(name="psA", bufs=2, space="PSUM") as psA,
                tc.tile_pool(name="psB", bufs=2, space="PSUM") as psB,
                tc.tile_pool(name="psT", bufs=1, space="PSUM") as psT,
            ):
                # K|Q|pm packed pass (stationary cols 0-31 = wk,
                # 32-63 = wq, 64 = wm) interleaved with the V projection
                # 4 j-blocks per t-chunk: matches the x DMA arrival pace
                # so the PE never idles during the input stream.
                for t in range(8):
                    kq = psA.tile([2 * D + 1, 512], F32, tag="kq_ps")
                    for cb in range(2):
                        nc.tensor.matmul(
                            kq[:],
                            wkq[:, cb, :],
                            xfs[cb][:, bass.ts(t, 512)],
                            start=(cb == 0),
                            stop=(cb == 1),
                        )
                    nc.scalar.copy(k_sb[0:D, bass.ts(t, 512)], kq[0:D, :])
                    # q staging copy; its free-axis accum gives the q colsum
                    nc.scalar.activation(
                        qt_sb[D : 2 * D, bass.ts(t, 512)],
                        kq[D : 2 * D, :],
                        AF.Copy,
                        accum_out=qpart[D : 2 * D, t : t + 1],
                    )
                    # mask logits: exp straight out of PSUM, with the
                    # softmax denominator accumulating for free
                    nc.scalar.activation(
                        emask_row[0:1, bass.ts(t, 512)],
                        kq[2 * D : 2 * D + 1, :],
                        AF.Exp,
                        accum_out=ztpart[0:1, t : t + 1],
                    )
                    # V^T[n, c] = relu(xf^T @ wv^T + bv), j-block-major
                    for jb in range(4 * t, 4 * t + 4):
                        vp = psB.tile([128, C], F32, tag="v_ps")
                        for cb in range(2):
                            nc.tensor.matmul(
                                vp[:],
                                xfs[cb][:, bass.ts(jb, 128)],
                                wvt[:, cb, :],
                                start=(cb == 0),
                                stop=False,
                            )
                        nc.tensor.matmul(
                            vp[:], ones_row[:], bv[:], start=False, stop=True
                        )
                        nc.vector.tensor_scalar_max(
                            vt_sb[:, bass.ts(jb, C)], vp[:], 0.0
                        )

                # center q over n:  qc = q - mean_n(q)  (rows 32-63)
                nc.vector.reduce_sum(
                    qsum[D : 2 * D], qpart[D : 2 * D, :], axis=mybir.AxisListType.X
                )
                nc.scalar.mul(qneg[D : 2 * D], qsum[D : 2 * D], -1.0 / N)
                nc.vector.tensor_scalar_add(
                    qct[D : 2 * D, :], qt_sb[D : 2 * D, 0:M], qneg[D : 2 * D]
                )
                # replicate K and Qc to the other 32-row group for the
                # 2-way row-packed E^T matmuls
                nc.sync.dma_start(k_sb[D : 2 * D, :], k_sb[0:D, :])
                nc.sync.dma_start(qct[0:D, :], qct[D : 2 * D, :])

                # unnormalized mask as a column tile [128, NB]: 32 tiny PE
                # transposes of [1, 128] slices (internal DRAM tiles fail
                # to load on this runtime, so no DRAM round trip)
                mcp = psT.tile([128, NB], F32, tag="mc_ps")
                for jb in range(NB):
                    nc.tensor.transpose(
                        mcp[:, jb : jb + 1],
                        emask_row[0:1, bass.ts(jb, 128)],
                        ones_rowf[0:1, 0:1],
                    )
                nc.vector.tensor_copy(mask_col[:], mcp[:])
                nc.vector.reduce_sum(
                    ztt[:], ztpart[:], axis=mybir.AxisListType.X
                )
                nc.vector.reciprocal_approx_fast(rz[:], ztt[:])
                nc.gpsimd.partition_broadcast(rzc[:], rz[:])

            # ---------------- main loop: attention ----------------
            with (
                tc.tile_pool(name="psE", bufs=2, space="PSUM") as psE,
                tc.tile_pool(name="psO", bufs=1, space="PSUM") as psO,
                tc.tile_pool(name="psS", bufs=1, space="PSUM") as psS,
                tc.tile_pool(name="psVM", bufs=1, space="PSUM") as psVM,
            ):

                def emit_e(mc, g):
                    # two K=32 matmuls packed into row groups (0,0)/(32,0);
                    # they run concurrently in the PE array
                    e_ps = psE.tile([128, 1024], F32, tag="e_ps", name="e_ps")
                    for h in range(2):
                        jb = 2 * g + h
                        nc.tensor.matmul(
                            e_ps[:, bass.ts(h, 512)],
                            k_sb[h * D : (h + 1) * D, bass.ts(jb, 128)],
                            qct[h * D : (h + 1) * D, bass.ts(mc, 512)],
                            start=True,
                            stop=True,
                            tile_position=(h * D, 0),
                        )
                    return e_ps

                def emit_vm():
                    # rank-1 mask correction, accumulated directly as
                    # per-partition columns (N=1 matmuls, no transposes)
                    vm_ps = psVM.tile([128, 2], F32, tag="vm_ps")
                    for jb in range(NB):
                        for h in range(2):
                            # one accumulation group for the whole
                            # bank: a second start=True on the same bank
                            # clears the other column's first contribution
                            nc.tensor.matmul(
                                vm_ps[:, h : h + 1],
                                vt_sb[:, jb * C + h * 128 : jb * C + (h + 1) * 128],
                                mask_col[:, jb : jb + 1],
                                start=(jb == 0 and h == 0),
                                stop=(jb == NB - 1 and h == 1),
                            )
                    nc.vector.tensor_scalar_mul(vm_col[:], vm_ps[:], rzc[:, 0:1])

                def emit_tail(mc, orw0, orw1, s_sb):
                    # normalize + mask add + store; no PE instructions
                    rs = npool.tile([1, 512], F32, tag="rs", name="rs")
                    nc.vector.reciprocal_approx_fast(rs[:], s_sb[:])
                    rb = npool.tile([128, 512], F32, tag="rb", name="rb")
                    nc.gpsimd.partition_broadcast(rb[:], rs[:])
                    for ci, orw in ((0, orw0), (1, orw1)):
                        t_sb = opool.tile([128, 512], F32, tag="t_sb", name="t_sb")
                        nc.vector.tensor_mul(t_sb[:], orw[:], rb[:])
                        o_sb = opool.tile([128, 512], F32, tag="o_sb", name="o_sb")
                        nc.vector.tensor_scalar_add(
                            o_sb[:], t_sb[:], vm_col[:, ci : ci + 1]
                        )
                        nc.sync.dma_start(
                            out_d[ci * 128 : (ci + 1) * 128, bass.ts(mc, 512)],
                            o_sb[:],
                        )

                e_pending = emit_e(0, 0)
                tail_args = None
                for mc in range(NMC):
                    o_ps = [
                        psO.tile([128, 512], F32, tag=f"o_ps{ci}", name=f"o_ps{ci}")
                        for ci in range(2)
                    ]
                    s_ps = psS.tile([1, 512], F32, tag="s_ps", name="s_ps")
                    for g in range(NG):
                        e_sb = epool.tile([128, 1024], BF16, tag="e_sb", name="e_sb")
                        nc.scalar.activation(e_sb[:], e_pending[:], AF.Exp)
                        if g + 1 < NG:
                            e_pending = emit_e(mc, g + 1)
                        elif mc + 1 < NMC:
                            e_pending = emit_e(mc + 1, 0)
                        for h in range(2):
                            jb = 2 * g + h
                            first = jb == 0
                            last = jb == NB - 1
                            e_half = e_sb[:, bass.ts(h, 512)]
                            for ci, (c0, cn) in enumerate(C_SPLITS):
                                nc.tensor.matmul(
                                    o_ps[ci][0:cn, :],
                                    vt_sb[:, jb * C + c0 : jb * C + c0 + cn],
                                    e_half,
                                    start=first,
                                    stop=last,
                                )
                            nc.tensor.matmul(
                                s_ps[:],
                                ones_col[:, 0:1],
                                e_half,
                                start=first,
                                stop=last,
                            )
                        if mc == 1 and g == 0:
                            emit_vm()
                        if g == 1 and tail_args is not None:
                            emit_tail(*tail_args)
                            tail_args = None
                    # eager PSUM-freeing copies so the next chunk's first
                    # accumulations never wait on the (lazy) tail
                    orw0 = opool.tile([128, 512], F32, tag="o_raw0", name="o_raw0")
                    nc.vector.tensor_copy(orw0[:], o_ps[0][:])
                    orw1 = opool.tile([128, 512], F32, tag="o_raw1", name="o_raw1")
                    nc.vector.tensor_copy(orw1[:], o_ps[1][:])
                    s_sb = npool.tile([1, 512], F32, tag="s_sb", name="s_sb")
                    nc.scalar.copy(s_sb[:], s_ps[:])
                    tail_args = (mc, orw0, orw1, s_sb)
                emit_tail(*tail_args)

    nc.compile()
    return nc


_NC_CACHE = {}


def _get_nc():
    if "nc" not in _NC_CACHE:
        _NC_CACHE["nc"] = build_nc()
    return _NC_CACHE["nc"]


def build_in_maps(x, wq, bq, wk, bk, wv, bv, wm, bm):
    x = np.ascontiguousarray(np.asarray(x, dtype=np.float32))
    xf = x.reshape(B, C, N)
    wq = np.asarray(wq, np.float32)
    wk = np.asarray(wk, np.float32)
    wv = np.asarray(wv, np.float32)
    wm = np.asarray(wm, np.float32)
    # packed K|Q|pm stationary: [p, cb, 0:32]=wk.T block, [32:64]=wq.T,
    # [64]=wm.T
    wkq = np.ascontiguousarray(
        np.concatenate(
            [
                wk.T.reshape(2, 128, D),
                wq.T.reshape(2, 128, D),
                wm.T.reshape(2, 128, 1),
            ],
            axis=2,
        ).transpose(1, 0, 2)
    )
    wvt = np.ascontiguousarray(wv.T.reshape(2, 128, C).transpose(1, 0, 2))
    bv2 = np.ascontiguousarray(np.asarray(bv, np.float32).reshape(1, C))

    in_maps = []
    for core in range(N_CORES):
        b, half = divmod(core, 2)
        if half == 0:
            xin = xf[b]
        else:
            # own query half first; j-sums are permutation invariant
            xin = np.concatenate([xf[b][:, M:], xf[b][:, :M]], axis=1)
        in_maps.append(
            {
                "x": np.ascontiguousarray(xin),
                "wkq": wkq,
                "wvt": wvt,
                "bv": bv2,
            }
        )
    return x, in_maps


def kernel(x, wq, bq, wk, bk, wv, bv, wm, bm):
    x, in_maps = build_in_maps(x, wq, bq, wk, bk, wv, bv, wm, bm)

    res = run_bass_kernel_spmd(_get_nc(), in_maps, list(range(N_CORES)))
    _NC_CACHE["last_results"] = res

    tissue = np.empty((B, C, N), np.float32)
    for core in range(N_CORES):
        b, half = divmod(core, 2)
        tissue[b][:, half * M : (half + 1) * M] = res.results[core]["out"]
    return x, tissue.reshape(B, C, H, W)
